# revision 15
# baseline (speedup 1.0000x reference)
"""DeepFFM Trainium2 kernel (8-core SPMD, batch-parallel).

Strategy
--------
All table lookups (FM_W / embedding; indices are data) and the statically
indexed FM_V slice are index-driven data staging, done on the host while
sharding.  The dense compute — the pairwise-interaction bilinear form, the
four-layer MLP, and the final sigmoid — runs on 8 NeuronCores, data-parallel
over the batch (2048 rows/core).

The MLP runs entirely in "transposed activation" layout: activations live as
[hidden, batch_tile] with hidden on partitions, so every layer is a plain
lhsT.T @ rhs matmul chain with no transposes.  The FFM bilinear term
v^T (S*mask) v is one bf16 matmul plus one DVE multiply; the first-order
(linear) term is folded on the host into one extra reduction row.  Everything
is packed into two constant DMAs plus two input DMAs per batch tile to keep
descriptor-generation off the critical path.
"""

import sys
import types

import numpy as np
import ml_dtypes


def _ensure_concourse():
    try:
        import concourse  # noqa: F401
    except ImportError:
        for p in ("/opt/trn_rl_repo", "/root/.axon_site/_ro/trn_rl_repo"):
            sys.path.insert(0, p)


def _ensure_axon_hooks_importable():
    """bass_utils imports antenv.axon_hooks unconditionally when tracing is
    requested; some images lack that module.  Provide a no-op registry so a
    trace request degrades gracefully instead of crashing."""
    try:
        import antenv
    except ImportError:
        return
    try:
        import antenv.axon_hooks  # noqa: F401
        return
    except ImportError:
        pass
    mod = types.ModuleType("antenv.axon_hooks")
    _hook = [None]
    mod.set_axon_ntff_profile_hook = lambda h: _hook.__setitem__(0, h)
    mod.get_axon_ntff_profile_hook = lambda: _hook[0]
    sys.modules["antenv.axon_hooks"] = mod
    antenv.axon_hooks = mod


_ensure_concourse()
_ensure_axon_hooks_importable()

import concourse.bass as bass  # noqa: E402
import concourse.tile as tile  # noqa: E402
from concourse import bacc, mybir  # noqa: E402
from concourse.bass import ds, ts  # noqa: E402
from concourse.bass_utils import run_bass_kernel_spmd  # noqa: E402
from concourse.tile_rust import add_dep_helper  # noqa: E402

F32 = mybir.dt.float32
BF16 = mybir.dt.bfloat16
AF = mybir.ActivationFunctionType
ALU = mybir.AluOpType
BF16NP = ml_dtypes.bfloat16

# Problem constants (fixed by the model definition).
FIELD_SIZE = 39
FEATURE_SIZE = 50000
N_GROUPS = 8
EMB = 8
D0 = FIELD_SIZE * EMB  # 312
N_CORES = 8
FIELD2FEATURE = np.arange(FIELD_SIZE, dtype=np.int64) * 1000
FIELD2FIELDS = np.arange(FIELD_SIZE, dtype=np.int64) % N_GROUPS

NT_COLS = 512  # batch columns per compute tile (one PSUM bank)
HALF = NT_COLS // 2
N_WARMUP = 7  # scratch matmuls at kernel start to ramp the PE clock (HAM)

FAUG = FIELD_SIZE + 2  # vals rows + linear-term row + ones row

# Column offsets of each weight block inside the packed [128, WPACK] blob.
_OFF_W0 = 0  # 3 chunks of 512
_OFF_W1 = 3 * 512  # 4 chunks of 256
_OFF_W2 = _OFF_W1 + 4 * 256  # 2 chunks of 128
_OFF_W3 = _OFF_W2 + 2 * 128  # 1 col
_OFF_MM = _OFF_W3 + 1  # FAUG cols (rows 0:FAUG)
WPACK = _OFF_MM + FAUG

_CACHE = {}


def _build_nc(bc: int):
    """Build + compile the per-core Bass program for a batch shard of `bc`."""
    nt = bc // NT_COLS
    nc = bacc.Bacc("TRN2", target_bir_lowering=False, debug=False)

    xp = nc.dram_tensor("xp", [nt, 128, 3, NT_COLS], BF16, kind="ExternalInput").ap()
    vw = nc.dram_tensor("vw", [128, bc], BF16, kind="ExternalInput").ap()
    wpA0 = nc.dram_tensor("wpA0", [128, 512 + 14], BF16, kind="ExternalInput").ap()
    wpA12 = nc.dram_tensor("wpA12", [128, 1024], BF16, kind="ExternalInput").ap()
    wpB = nc.dram_tensor("wpB", [128, WPACK - _OFF_W1], BF16, kind="ExternalInput").ap()
    out = nc.dram_tensor("out", [1, bc], F32, kind="ExternalOutput").ap()

    with tile.TileContext(nc) as tc:
        with (
            tc.tile_pool(name="consts", bufs=1) as consts,
            tc.tile_pool(name="iox", bufs=1) as iox,
            tc.tile_pool(name="acts", bufs=2) as acts,
            tc.tile_pool(name="small", bufs=3) as small,
            tc.tile_pool(name="warm", bufs=1) as warm,
            tc.tile_pool(name="psA", bufs=4, space="PSUM") as psA,
            tc.tile_pool(name="psB", bufs=3, space="PSUM") as psB,
            tc.tile_pool(name="psC", bufs=1, space="PSUM") as psC,
        ):
            # ---- input loads, in need-order (DMA queues serve concurrent
            # transfers round-robin; issue order biases completion order) ----
            wA0 = consts.tile([128, 512 + 14], BF16, tag="wA0")
            nc.sync.dma_start(wA0, wpA0)
            x0a = iox.tile([128, 1, NT_COLS], BF16, tag="x0a")
            nc.sync.dma_start(x0a, xp[0][:, 0:1])
            wA12 = consts.tile([128, 1024], BF16, tag="wA12")
            nc.sync.dma_start(wA12, wpA12)
            x0bc = iox.tile([128, 2, NT_COLS], BF16, tag="x0bc")
            nc.sync.dma_start(x0bc, xp[0][:, 1:3])
            wB = consts.tile([128, WPACK - _OFF_W1], BF16, tag="wB")
            nc.sync.dma_start(wB, wpB)
            vw_all = consts.tile([128, bc], BF16, tag="vw")
            nc.sync.dma_start(vw_all, vw)
            xts = [None] * nt
            for i in range(1, nt):
                xts[i] = iox.tile(
                    [128, 3, NT_COLS], BF16, tag=f"x{i}", name=f"xt{i}"
                )
                nc.sync.dma_start(xts[i], xp[i])

            # ---- HAM warmup: keep the PE busy while DMAs land so the
            # clock ramps to 2.4 GHz before the first real matmul ----
            wscr = warm.tile([128, 128], BF16, tag="wscr")
            nc.vector.memset(wscr, 0.0)
            xscr = warm.tile([128, NT_COLS], BF16, tag="xscr")
            nc.vector.memset(xscr, 0.0)
            pscr = psA.tile([128, NT_COLS], F32, tag="pmlp", name="pscr")
            for _ in range(N_WARMUP):
                nc.tensor.matmul(pscr, wscr, xscr, start=True, stop=True)

            ones = consts.tile([FAUG, 1], BF16, tag="ones")
            nc.vector.memset(ones, 1.0)

            def w0k(k):  # lhsT [K, 512] chunk k of deepW0
                if k == 0:
                    return wA0[:, 0:512]
                if k == 1:
                    return wA12[:, 0:512]
                return wA12[0:56, 512:1024]

            fpt = wA0[:, 512:526].bitcast(F32)
            b0t = fpt[:, 0:4]
            b1t = fpt[:, 4:6]
            b2t = fpt[:, 6:7]

            def relu_halves(dst, p, bias_ap):
                nc.scalar.activation(
                    dst[:, 0:HALF], p[:, 0:HALF], AF.Relu, bias=bias_ap, scale=1.0
                )
                nc.vector.tensor_scalar(
                    dst[:, HALF:NT_COLS],
                    p[:, HALF:NT_COLS],
                    bias_ap,
                    0.0,
                    ALU.add,
                    ALU.max,
                )

            # ---- batch tiles ----
            for t_i in range(nt):
                cols = ts(t_i, NT_COLS)
                if t_i == 0:
                    xin = (x0a[:, 0, :], x0bc[:, 0, :], x0bc[0:56, 1, :])
                else:
                    xt = xts[t_i]
                    xin = (xt[:, 0, :], xt[:, 1, :], xt[0:56, 2, :])
                vwt = vw_all[0:FAUG, cols]

                # layer 1: 312 -> 512, k-major so compute starts on chunk 0
                ps1 = [
                    psA.tile([128, NT_COLS], F32, tag="pmlp", name=f"p1_{t_i}_{m}")
                    for m in range(4)
                ]
                for k in range(3):
                    for m in range(4):
                        nc.tensor.matmul(
                            ps1[m],
                            w0k(k)[:, ds(m * 128, 128)],
                            xin[k],
                            start=(k == 0),
                            stop=(k == 2),
                            skip_group_check=True,
                        )
                h1 = acts.tile([128, 4, NT_COLS], BF16, tag="h1")
                for m in range(4):
                    relu_halves(h1[:, m], ps1[m], b0t[:, ds(m, 1)])

                # layer 2: 512 -> 256 (2 output chunks, m-major)
                h2 = acts.tile([128, 2, NT_COLS], BF16, tag="h2")
                for m in range(2):
                    p = psA.tile([128, NT_COLS], F32, tag="pmlp", name=f"p2_{t_i}_{m}")
                    for k in range(4):
                        nc.tensor.matmul(
                            p,
                            wB[:, ds(k * 256 + m * 128, 128)],
                            h1[:, k],
                            start=(k == 0),
                            stop=(k == 3),
                        )
                    relu_halves(h2[:, m], p, b1t[:, ds(m, 1)])

                # layer 3: 256 -> 128
                h3 = acts.tile([128, NT_COLS], BF16, tag="h3")
                p = psA.tile([128, NT_COLS], F32, tag="pmlp", name=f"p3_{t_i}")
                for k in range(2):
                    nc.tensor.matmul(
                        p, wB[:, ds(_OFF_W2 - _OFF_W1 + k * 128, 128)], h2[:, k],
                        start=(k == 0), stop=(k == 1),
                    )
                relu_halves(h3, p, b2t)

                # FFM + linear: v_aug = [vals; lin; 1], M_aug routes lin
                # through Y[39,:]==1 so colsum(Y*v_aug) = inter + linear.
                pf = psB.tile([FAUG, NT_COLS], F32, tag="pffm")
                nc.tensor.matmul(
                    pf,
                    wB[0:FAUG, ds(_OFF_MM - _OFF_W1, FAUG)],
                    vwt,
                    start=True,
                    stop=True,
                )
                r = small.tile([FAUG, NT_COLS], BF16, tag="r")
                nc.vector.tensor_mul(r, pf, vwt)

                # head: deep_out + colsum(r) -> sigmoid
                po = psC.tile([1, NT_COLS], F32, tag="pout")
                nc.tensor.matmul(
                    po, wB[:, ds(_OFF_W3 - _OFF_W1, 1)], h3, start=True, stop=False
                )
                nc.tensor.matmul(po, ones, r, start=False, stop=True)
                o_sb = small.tile([1, NT_COLS], F32, tag="o")
                nc.scalar.activation(o_sb, po, AF.Sigmoid, scale=1.0)
                nc.sync.dma_start(out[:, cols], o_sb)

    nc.compile()
    return nc


def _prep_host(inputs):
    """Index-driven staging + layout prep on the host; returns per-core maps."""
    feat_ids = np.asarray(inputs["feat_ids"], dtype=np.int64)
    feat_vals = np.ascontiguousarray(np.asarray(inputs["feat_vals"], dtype=np.float32))
    FM_W = np.asarray(inputs["FM_W"], dtype=np.float32)
    FM_V = np.asarray(inputs["FM_V"])
    FM_B = np.asarray(inputs["FM_B"], dtype=np.float32)
    embedding = np.asarray(inputs["embedding"], dtype=np.float32)
    outW = np.asarray(inputs["outW"], dtype=np.float32)
    outB = np.asarray(inputs["outB"], dtype=np.float32)

    B = feat_ids.shape[0]
    assert B % N_CORES == 0
    bc = B // N_CORES
    assert bc % NT_COLS == 0

    # Pairwise-interaction matrix: only 39 statically indexed rows of FM_V.
    Vi = np.stack(
        [
            np.asarray(FM_V[i, FIELD2FEATURE[i]], dtype=np.float32)
            for i in range(FIELD_SIZE)
        ]
    )  # [F, G, E]
    Vg = Vi[:, FIELD2FIELDS, :]  # [F, F, E]
    S = np.einsum("ije,jie->ij", Vg, Vg).astype(np.float32)
    M = S * np.triu(np.ones((FIELD_SIZE, FIELD_SIZE), np.float32), k=1)

    # Gathers (host staging) and transposed layouts.
    XT = embedding[feat_ids].reshape(B, D0).T.astype(BF16NP)  # [312, B]
    nt_total = B // NT_COLS
    xp = np.zeros((nt_total, 128, 3, NT_COLS), dtype=BF16NP)
    xv = XT.reshape(312, nt_total, NT_COLS)
    xp[:, :, 0, :] = xv[0:128].transpose(1, 0, 2)
    xp[:, :, 1, :] = xv[128:256].transpose(1, 0, 2)
    xp[:, 0:56, 2, :] = xv[256:312].transpose(1, 0, 2)

    lin = (FM_W[feat_ids] * feat_vals).sum(axis=1) + (
        FM_B.reshape(-1)[0] + outB.reshape(-1)[0]
    )  # [B]
    vw = np.zeros((128, B), dtype=BF16NP)
    vw[0:FIELD_SIZE] = feat_vals.T.astype(BF16NP)
    vw[FIELD_SIZE] = lin.astype(BF16NP)
    vw[FIELD_SIZE + 1] = np.ones((B,), dtype=BF16NP)

    # Weight pack [128, WPACK] bf16: w0 chunks | w1 chunks | w2 chunks | w3 | M
    wpack = np.zeros((128, WPACK), dtype=BF16NP)
    w0 = np.asarray(inputs["deepW0"], dtype=np.float32).astype(BF16NP)
    for k, kk in enumerate((128, 128, 56)):
        wpack[0:kk, _OFF_W0 + k * 512 : _OFF_W0 + (k + 1) * 512] = w0[
            k * 128 : k * 128 + kk
        ]
    w1 = np.asarray(inputs["deepW1"], dtype=np.float32).astype(BF16NP)
    for k in range(4):
        wpack[:, _OFF_W1 + k * 256 : _OFF_W1 + (k + 1) * 256] = w1[
            k * 128 : (k + 1) * 128
        ]
    w2 = np.asarray(inputs["deepW2"], dtype=np.float32).astype(BF16NP)
    for k in range(2):
        wpack[:, _OFF_W2 + k * 128 : _OFF_W2 + (k + 1) * 128] = w2[
            k * 128 : (k + 1) * 128
        ]
    wpack[:, _OFF_W3 : _OFF_W3 + 1] = outW.astype(BF16NP)
    M_aug = np.zeros((FAUG, FAUG), dtype=np.float32)
    M_aug[0:FIELD_SIZE, 0:FIELD_SIZE] = M
    M_aug[FIELD_SIZE + 1, FIELD_SIZE] = 1.0  # Y[39,:] = 1 -> routes lin row
    wpack[0:FAUG, _OFF_MM : _OFF_MM + FAUG] = M_aug.astype(BF16NP)
    fpk = np.zeros((128, 7), dtype=np.float32)
    fpk[:, 0:4] = np.asarray(inputs["deepB0"], dtype=np.float32).reshape(4, 128).T
    fpk[:, 4:6] = np.asarray(inputs["deepB1"], dtype=np.float32).reshape(2, 128).T
    fpk[:, 6:7] = np.asarray(inputs["deepB2"], dtype=np.float32).reshape(1, 128).T
    wpA0 = np.ascontiguousarray(
        np.concatenate([wpack[:, 0:512], fpk.view(BF16NP)], axis=1)
    )
    wpA12 = np.ascontiguousarray(wpack[:, 512:_OFF_W1])
    wpB = np.ascontiguousarray(wpack[:, _OFF_W1:])

    shared = dict(wpA0=wpA0, wpA12=wpA12, wpB=wpB)
    in_maps = []
    for c in range(N_CORES):
        cols = slice(c * bc, (c + 1) * bc)
        m = dict(shared)
        nt_c = bc // NT_COLS
        m["xp"] = np.ascontiguousarray(xp[c * nt_c : (c + 1) * nt_c])
        m["vw"] = np.ascontiguousarray(vw[:, cols])
        in_maps.append(m)
    return in_maps, bc


def _run(inputs, trace=False, **kwargs):
    in_maps, bc = _prep_host(inputs)
    if bc not in _CACHE:
        _CACHE[bc] = _build_nc(bc)
    nc = _CACHE[bc]
    res = run_bass_kernel_spmd(
        nc, in_maps, core_ids=list(range(N_CORES)), trace=trace, **kwargs
    )
    out = np.concatenate(
        [np.asarray(res.results[c]["out"], dtype=np.float32)[0] for c in range(N_CORES)]
    )
    return out, res


def kernel(**inputs) -> np.ndarray:
    out, _ = _run(inputs)
    return out


# revision 16
# speedup vs baseline: 1.1399x; 1.1399x over previous
"""DeepFFM Trainium2 kernel (8-core SPMD, batch-parallel).

Strategy
--------
All table lookups (FM_W / embedding; indices are data) and the statically
indexed FM_V slice are index-driven data staging, done on the host while
sharding.  The dense compute — the pairwise-interaction bilinear form, the
four-layer MLP, and the final sigmoid — runs on 8 NeuronCores, data-parallel
over the batch (2048 rows/core).

The MLP runs entirely in "transposed activation" layout: activations live as
[hidden, batch_tile] with hidden on partitions, so every layer is a plain
lhsT.T @ rhs matmul chain with no transposes.  The FFM bilinear term
v^T (S*mask) v is one bf16 matmul plus one DVE multiply; the first-order
(linear) term is folded on the host into one extra reduction row.  Everything
is packed into two constant DMAs plus two input DMAs per batch tile to keep
descriptor-generation off the critical path.
"""

import sys
import types

import numpy as np
import ml_dtypes


def _ensure_concourse():
    try:
        import concourse  # noqa: F401
    except ImportError:
        for p in ("/opt/trn_rl_repo", "/root/.axon_site/_ro/trn_rl_repo"):
            sys.path.insert(0, p)


def _ensure_axon_hooks_importable():
    """bass_utils imports antenv.axon_hooks unconditionally when tracing is
    requested; some images lack that module.  Provide a no-op registry so a
    trace request degrades gracefully instead of crashing."""
    try:
        import antenv
    except ImportError:
        return
    try:
        import antenv.axon_hooks  # noqa: F401
        return
    except ImportError:
        pass
    mod = types.ModuleType("antenv.axon_hooks")
    _hook = [None]
    mod.set_axon_ntff_profile_hook = lambda h: _hook.__setitem__(0, h)
    mod.get_axon_ntff_profile_hook = lambda: _hook[0]
    sys.modules["antenv.axon_hooks"] = mod
    antenv.axon_hooks = mod


_ensure_concourse()
_ensure_axon_hooks_importable()

import concourse.bass as bass  # noqa: E402
import concourse.tile as tile  # noqa: E402
from concourse import bacc, mybir  # noqa: E402
from concourse.bass import ds, ts  # noqa: E402
from concourse.bass_utils import run_bass_kernel_spmd  # noqa: E402
from concourse.tile_rust import add_dep_helper  # noqa: E402

F32 = mybir.dt.float32
BF16 = mybir.dt.bfloat16
AF = mybir.ActivationFunctionType
ALU = mybir.AluOpType
BF16NP = ml_dtypes.bfloat16

# Problem constants (fixed by the model definition).
FIELD_SIZE = 39
FEATURE_SIZE = 50000
N_GROUPS = 8
EMB = 8
D0 = FIELD_SIZE * EMB  # 312
N_CORES = 8
FIELD2FEATURE = np.arange(FIELD_SIZE, dtype=np.int64) * 1000
FIELD2FIELDS = np.arange(FIELD_SIZE, dtype=np.int64) % N_GROUPS

NT_COLS = 512  # batch columns per compute tile (one PSUM bank)
HALF = NT_COLS // 2
N_WARMUP = 10  # scratch matmuls at kernel start to ramp the PE clock (HAM)

FAUG = FIELD_SIZE + 2  # vals rows + linear-term row + ones row

# Column offsets of each weight block inside the packed [128, WPACK] blob.
_OFF_W0 = 0  # 3 chunks of 512
_OFF_W1 = 3 * 512  # 4 chunks of 256
_OFF_W2 = _OFF_W1 + 4 * 256  # 2 chunks of 128
_OFF_W3 = _OFF_W2 + 2 * 128  # 1 col
_OFF_MM = _OFF_W3 + 1  # FAUG cols (rows 0:FAUG)
WPACK = _OFF_MM + FAUG

_CACHE = {}


def _build_nc(bc: int):
    """Build + compile the per-core Bass program for a batch shard of `bc`."""
    nt = bc // NT_COLS
    nc = bacc.Bacc("TRN2", target_bir_lowering=False, debug=False)

    xp = nc.dram_tensor("xp", [nt, 128, 3, NT_COLS], BF16, kind="ExternalInput").ap()
    vw = nc.dram_tensor("vw", [128, bc], BF16, kind="ExternalInput").ap()
    wpA = nc.dram_tensor("wpA", [128, _OFF_W1 + 14], BF16, kind="ExternalInput").ap()
    wpB = nc.dram_tensor("wpB", [128, WPACK - _OFF_W1], BF16, kind="ExternalInput").ap()
    out = nc.dram_tensor("out", [1, bc], F32, kind="ExternalOutput").ap()

    with tile.TileContext(nc) as tc:
        with (
            tc.tile_pool(name="consts", bufs=1) as consts,
            tc.tile_pool(name="iox", bufs=1) as iox,
            tc.tile_pool(name="acts", bufs=3) as acts,
            tc.tile_pool(name="small", bufs=3) as small,
            tc.tile_pool(name="warm", bufs=1) as warm,
            tc.tile_pool(name="psA", bufs=5, space="PSUM") as psA,
            tc.tile_pool(name="psB", bufs=2, space="PSUM") as psB,
            tc.tile_pool(name="psC", bufs=1, space="PSUM") as psC,
        ):
            # ---- input loads, in need-order (DMA queues serve concurrent
            # transfers round-robin; issue order biases completion order) ----
            wA = consts.tile([128, _OFF_W1 + 14], BF16, tag="wA")
            nc.sync.dma_start(wA, wpA)
            xts = [
                iox.tile([128, 3, NT_COLS], BF16, tag=f"x{i}", name=f"xt{i}")
                for i in range(nt)
            ]
            nc.sync.dma_start(xts[0], xp[0])
            wB = consts.tile([128, WPACK - _OFF_W1], BF16, tag="wB")
            nc.sync.dma_start(wB, wpB)
            vw_all = consts.tile([128, bc], BF16, tag="vw")
            nc.sync.dma_start(vw_all, vw)
            for i in range(1, nt):
                nc.sync.dma_start(xts[i], xp[i])

            # ---- HAM warmup: keep the PE busy while DMAs land so the
            # clock ramps to 2.4 GHz before the first real matmul ----
            wscr = warm.tile([128, 128], BF16, tag="wscr")
            nc.vector.memset(wscr, 0.0)
            xscr = warm.tile([128, NT_COLS], BF16, tag="xscr")
            nc.vector.memset(xscr, 0.0)
            pscr = psA.tile([128, NT_COLS], F32, tag="pmlp", name="pscr")
            for _ in range(N_WARMUP):
                nc.tensor.matmul(pscr, wscr, xscr, start=True, stop=True)

            ones = consts.tile([FAUG, 1], BF16, tag="ones")
            nc.vector.memset(ones, 1.0)

            def w0k(k):  # lhsT [K, 512] chunk k of deepW0
                kk = 56 if k == 2 else 128
                return wA[0:kk, ts(k, 512)]

            fpt = wA[:, _OFF_W1 : _OFF_W1 + 14].bitcast(F32)
            b0t = fpt[:, 0:4]
            b1t = fpt[:, 4:6]
            b2t = fpt[:, 6:7]

            def relu_full(dst, p, bias_ap, on_vector):
                if on_vector:
                    nc.vector.tensor_scalar(dst, p, bias_ap, 0.0, ALU.add, ALU.max)
                else:
                    nc.scalar.activation(dst, p, AF.Relu, bias=bias_ap, scale=1.0)

            # ---- batch tiles ----
            for t_i in range(nt):
                cols = ts(t_i, NT_COLS)
                xt = xts[t_i]
                xin = (xt[:, 0, :], xt[:, 1, :], xt[0:56, 2, :])
                vwt = vw_all[0:FAUG, cols]

                # layer 1: 312 -> 512 (4 output chunks of 128)
                h1 = acts.tile([128, 4, NT_COLS], BF16, tag="h1")
                for m in range(4):
                    p = psA.tile([128, NT_COLS], F32, tag="pmlp", name=f"p1_{t_i}_{m}")
                    for k in range(3):
                        nc.tensor.matmul(
                            p,
                            w0k(k)[:, ds(m * 128, 128)],
                            xin[k],
                            start=(k == 0),
                            stop=(k == 2),
                        )
                    relu_full(h1[:, m], p, b0t[:, ds(m, 1)], on_vector=(m % 2 == 1))

                # layer 2: 512 -> 256 (2 output chunks)
                h2 = acts.tile([128, 2, NT_COLS], BF16, tag="h2")
                for m in range(2):
                    p = psA.tile([128, NT_COLS], F32, tag="pmlp", name=f"p2_{t_i}_{m}")
                    for k in range(4):
                        nc.tensor.matmul(
                            p,
                            wB[:, ds(k * 256 + m * 128, 128)],
                            h1[:, k],
                            start=(k == 0),
                            stop=(k == 3),
                        )
                    relu_full(h2[:, m], p, b1t[:, ds(m, 1)], on_vector=(m == 1))

                # layer 3: 256 -> 128
                h3 = acts.tile([128, NT_COLS], BF16, tag="h3")
                p = psA.tile([128, NT_COLS], F32, tag="pmlp", name=f"p3_{t_i}")
                for k in range(2):
                    nc.tensor.matmul(
                        p, wB[:, ds(_OFF_W2 - _OFF_W1 + k * 128, 128)], h2[:, k],
                        start=(k == 0), stop=(k == 1),
                    )
                relu_full(h3, p, b2t, on_vector=False)

                # FFM + linear: v_aug = [vals; lin; 1], M_aug routes lin
                # through Y[39,:]==1 so colsum(Y*v_aug) = inter + linear.
                pf = psB.tile([FAUG, NT_COLS], F32, tag="pffm")
                nc.tensor.matmul(
                    pf,
                    wB[0:FAUG, ds(_OFF_MM - _OFF_W1, FAUG)],
                    vwt,
                    start=True,
                    stop=True,
                )
                r = small.tile([FAUG, NT_COLS], BF16, tag="r")
                nc.vector.tensor_mul(r, pf, vwt)

                # head: deep_out + colsum(r) -> sigmoid
                po = psC.tile([1, NT_COLS], F32, tag="pout")
                nc.tensor.matmul(
                    po, wB[:, ds(_OFF_W3 - _OFF_W1, 1)], h3, start=True, stop=False
                )
                nc.tensor.matmul(po, ones, r, start=False, stop=True)
                o_sb = small.tile([1, NT_COLS], F32, tag="o")
                nc.scalar.activation(o_sb, po, AF.Sigmoid, scale=1.0)
                nc.sync.dma_start(out[:, cols], o_sb)

    nc.compile()
    return nc


def _prep_host(inputs):
    """Index-driven staging + layout prep on the host; returns per-core maps."""
    feat_ids = np.asarray(inputs["feat_ids"], dtype=np.int64)
    feat_vals = np.ascontiguousarray(np.asarray(inputs["feat_vals"], dtype=np.float32))
    FM_W = np.asarray(inputs["FM_W"], dtype=np.float32)
    FM_V = np.asarray(inputs["FM_V"])
    FM_B = np.asarray(inputs["FM_B"], dtype=np.float32)
    embedding = np.asarray(inputs["embedding"], dtype=np.float32)
    outW = np.asarray(inputs["outW"], dtype=np.float32)
    outB = np.asarray(inputs["outB"], dtype=np.float32)

    B = feat_ids.shape[0]
    assert B % N_CORES == 0
    bc = B // N_CORES
    assert bc % NT_COLS == 0

    # Pairwise-interaction matrix: only 39 statically indexed rows of FM_V.
    Vi = np.stack(
        [
            np.asarray(FM_V[i, FIELD2FEATURE[i]], dtype=np.float32)
            for i in range(FIELD_SIZE)
        ]
    )  # [F, G, E]
    Vg = Vi[:, FIELD2FIELDS, :]  # [F, F, E]
    S = np.einsum("ije,jie->ij", Vg, Vg).astype(np.float32)
    M = S * np.triu(np.ones((FIELD_SIZE, FIELD_SIZE), np.float32), k=1)

    # Gathers (host staging) and transposed layouts.
    XT = embedding[feat_ids].reshape(B, D0).T.astype(BF16NP)  # [312, B]
    nt_total = B // NT_COLS
    xp = np.zeros((nt_total, 128, 3, NT_COLS), dtype=BF16NP)
    xv = XT.reshape(312, nt_total, NT_COLS)
    xp[:, :, 0, :] = xv[0:128].transpose(1, 0, 2)
    xp[:, :, 1, :] = xv[128:256].transpose(1, 0, 2)
    xp[:, 0:56, 2, :] = xv[256:312].transpose(1, 0, 2)

    lin = (FM_W[feat_ids] * feat_vals).sum(axis=1) + (
        FM_B.reshape(-1)[0] + outB.reshape(-1)[0]
    )  # [B]
    vw = np.zeros((128, B), dtype=BF16NP)
    vw[0:FIELD_SIZE] = feat_vals.T.astype(BF16NP)
    vw[FIELD_SIZE] = lin.astype(BF16NP)
    vw[FIELD_SIZE + 1] = np.ones((B,), dtype=BF16NP)

    # Weight pack [128, WPACK] bf16: w0 chunks | w1 chunks | w2 chunks | w3 | M
    wpack = np.zeros((128, WPACK), dtype=BF16NP)
    w0 = np.asarray(inputs["deepW0"], dtype=np.float32).astype(BF16NP)
    for k, kk in enumerate((128, 128, 56)):
        wpack[0:kk, _OFF_W0 + k * 512 : _OFF_W0 + (k + 1) * 512] = w0[
            k * 128 : k * 128 + kk
        ]
    w1 = np.asarray(inputs["deepW1"], dtype=np.float32).astype(BF16NP)
    for k in range(4):
        wpack[:, _OFF_W1 + k * 256 : _OFF_W1 + (k + 1) * 256] = w1[
            k * 128 : (k + 1) * 128
        ]
    w2 = np.asarray(inputs["deepW2"], dtype=np.float32).astype(BF16NP)
    for k in range(2):
        wpack[:, _OFF_W2 + k * 128 : _OFF_W2 + (k + 1) * 128] = w2[
            k * 128 : (k + 1) * 128
        ]
    wpack[:, _OFF_W3 : _OFF_W3 + 1] = outW.astype(BF16NP)
    M_aug = np.zeros((FAUG, FAUG), dtype=np.float32)
    M_aug[0:FIELD_SIZE, 0:FIELD_SIZE] = M
    M_aug[FIELD_SIZE + 1, FIELD_SIZE] = 1.0  # Y[39,:] = 1 -> routes lin row
    wpack[0:FAUG, _OFF_MM : _OFF_MM + FAUG] = M_aug.astype(BF16NP)
    fpk = np.zeros((128, 7), dtype=np.float32)
    fpk[:, 0:4] = np.asarray(inputs["deepB0"], dtype=np.float32).reshape(4, 128).T
    fpk[:, 4:6] = np.asarray(inputs["deepB1"], dtype=np.float32).reshape(2, 128).T
    fpk[:, 6:7] = np.asarray(inputs["deepB2"], dtype=np.float32).reshape(1, 128).T
    wpA = np.ascontiguousarray(
        np.concatenate([wpack[:, :_OFF_W1], fpk.view(BF16NP)], axis=1)
    )
    wpB = np.ascontiguousarray(wpack[:, _OFF_W1:])

    shared = dict(wpA=wpA, wpB=wpB)
    in_maps = []
    for c in range(N_CORES):
        cols = slice(c * bc, (c + 1) * bc)
        m = dict(shared)
        nt_c = bc // NT_COLS
        m["xp"] = np.ascontiguousarray(xp[c * nt_c : (c + 1) * nt_c])
        m["vw"] = np.ascontiguousarray(vw[:, cols])
        in_maps.append(m)
    return in_maps, bc


def _run(inputs, trace=False, **kwargs):
    in_maps, bc = _prep_host(inputs)
    if bc not in _CACHE:
        _CACHE[bc] = _build_nc(bc)
    nc = _CACHE[bc]
    res = run_bass_kernel_spmd(
        nc, in_maps, core_ids=list(range(N_CORES)), trace=trace, **kwargs
    )
    out = np.concatenate(
        [np.asarray(res.results[c]["out"], dtype=np.float32)[0] for c in range(N_CORES)]
    )
    return out, res


def kernel(**inputs) -> np.ndarray:
    out, _ = _run(inputs)
    return out
